# revision 13
# baseline (speedup 1.0000x reference)
"""Chamfer distance (bidirectional, thresholded) on 8 Trainium2 NeuronCores.

Problem: source_pc/target_pc [2, 16384, 3] fp32 -> [2] fp32.
  dist[b,n,m] = ||src[b,n] - tgt[b,m]||
  out[b] = (mean_n min(min_m dist, T) + mean_m min(min_n dist, T)) / 2

Strategy
--------
Sharding: batch (2) x source-slice (4) over the 8 cores. Each core computes,
for its batch b and its 4096-point source slice:
  * dist1 partial: min_m d2(n, m) for its 4096 n's (complete min over all m)
  * dist2 partial: min_{n in slice} d2(n, m) for all 16384 m's
The host min-reduces dist2 partials across the 4 cores of a batch and applies
sqrt/threshold/mean (cheap: 16K values).

Device kernel: d2 is computed by the TensorEngine via an augmented inner
product of K=30 rows: d2 = x^2 + y^2 - 2xy, with each fp32 operand split into
3 bf16 chunks (exact 24-bit split; cross-chunk product rows give fp32-level
accuracy at bf16 PE speed - fp32 matmul is 4x slower). K=30 <= 32 enables 4x
row-tiling: 4 concurrent matmuls via tile_position=(32q, 0), each fed from
its own 32-partition SBUF quadrant (inputs quadrant-replicated on host).

The bottleneck is the min-reduction: 2*2*16384^2 = 1.07G d2 values must
leave PSUM, and VectorE is the only engine with a min-reduce (1 col/cycle;
the fused tensor_tensor_reduce and Pool-engine min are rejected by this
toolchain). To beat the DVE-only rate the kernel works on NEGATED d2 and
splits supersteps into two kinds:
  * direct: VectorE max-reduces the two PSUM tiles (max of -d2 = -min d2).
  * tournament: the PE additionally emits D' = d2(c_i) - d2(c_{i+1}) via a
    2-matmul PSUM accumulation; ScalarE computes relu(D') and copies A'
    (both PSUM->SBUF, line rate); GpSimd adds them (the only elementwise op
    walrus accepts on Pool): A' + relu(D') = max(-d2(c_i), -d2(c_{i+1})),
    halving what VectorE must reduce for those supersteps.
This spreads the drain across ScalarE + GpSimd + VectorE at ~1.3x the
DVE-only throughput.
"""

import numpy as np
import ml_dtypes

B = 2
N = 16384
M = 16384
CORES = 8
NSLICE = N // 4          # source points per core
T1 = NSLICE // 128       # 32 dist1 output tiles (n on partitions)
T2 = M // 128            # 128 dist2 output tiles (m on partitions)
SS1 = M // 2048          # 8 supersteps (of 4x512 streamed cols) per n-tile
SS2 = NSLICE // 2048     # 2 supersteps per m-tile
# supersteps whose two PSUM tiles VectorE max-reduces directly; the rest go
# through the ScalarE-relu / GpSimd-add tournament (1 partial instead of 2)
D1_DIRECT = 4
D2_DIRECT = 1
# pair two tournament supersteps into one wide [128,2048] GpSimd add + one
# wide VectorE reduce (GpSimd per-op overhead is the tournament binder)
PAIR_GP = True
def _npart(n_ss, direct):
    n_tourn = n_ss - direct
    if PAIR_GP and n_tourn % 2 == 0:
        return 2 * direct + n_tourn // 2
    # n_tourn == 1: paired across consecutive output-tiles instead
    return 2 * direct + n_tourn


NPART1 = _npart(SS1, D1_DIRECT)
NPART2 = _npart(SS2, D2_DIRECT)
KROWS = 30
THRESHOLD = 33.33

# chunk-index pairs (lhs_chunk, rhs_chunk); 0=hi 1=mid 2=lo. (2,2) dropped
# (contributes ~2^-34 relative - far below fp32 rounding of the sum).
_PAIRS = [(0, 0), (0, 1), (1, 0), (0, 2), (2, 0), (1, 1), (1, 2), (2, 1)]

_BF16 = ml_dtypes.bfloat16


def _split3(a):
    """Exact 3-way bf16 split of fp32: a == h + m + l (24-bit mantissa)."""
    h = a.astype(_BF16)
    r = a - h.astype(np.float32)
    m = r.astype(_BF16)
    r2 = r - m.astype(np.float32)
    l = r2.astype(_BF16)
    return h, m, l


def _forms(pts):
    """pts [n,3] fp32 -> (lhs_form, rhs_form), each [KROWS, n] bf16.

    sum_k lhs[k, i] * rhs'[k, j] (for rhs' built from another point set)
    = |p_i|^2 + |q_j|^2 - 2 p_i . q_j  (up to dropped (lo,lo) terms).
    """
    pts = np.ascontiguousarray(pts, dtype=np.float32)
    n = pts.shape[0]
    sq = np.sum(pts * pts, axis=1, dtype=np.float32)
    coord_l = [_split3(np.float32(-2.0) * pts[:, d]) for d in range(3)]
    coord_r = [_split3(pts[:, d]) for d in range(3)]
    sq_c = _split3(sq)
    ones = np.ones(n, dtype=_BF16)
    lhs = np.empty((KROWS, n), dtype=_BF16)
    rhs = np.empty((KROWS, n), dtype=_BF16)
    k = 0
    for d in range(3):
        for (i, j) in _PAIRS:
            lhs[k] = coord_l[d][i]
            rhs[k] = coord_r[d][j]
            k += 1
    for c in range(3):
        lhs[k] = sq_c[c]
        rhs[k] = ones
        k += 1
    for c in range(3):
        lhs[k] = ones
        rhs[k] = sq_c[c]
        k += 1
    assert k == KROWS
    return lhs, rhs


def _quad(a):
    """[KROWS, X] -> [128, X]: replicate into the 4 SBUF quadrants."""
    out = np.zeros((128, a.shape[1]), dtype=a.dtype)
    for q in range(4):
        out[q * 32: q * 32 + KROWS] = a
    return out


def _neg(a):
    """Exact bf16 negation."""
    return (-a.astype(np.float32)).astype(_BF16)


_NC_CACHE = {}


def build_bass(repeat=1):
    """Build (and cache) the single-core Bass/Tile program.

    repeat > 1 wraps the whole compute in an on-device loop; used by the
    test harness to amortize the ~88 ms axon dispatch floor when timing.
    """
    if repeat in _NC_CACHE:
        return _NC_CACHE[repeat]

    import concourse.tile as tile
    from concourse import bacc, mybir

    f32 = mybir.dt.float32
    bf16 = mybir.dt.bfloat16
    MAX = mybir.AluOpType.max
    ADD = mybir.AluOpType.add
    AXX = mybir.AxisListType.X
    RELU = mybir.ActivationFunctionType.Relu

    nc = bacc.Bacc(None, target_bir_lowering=False)
    srcLn_d = nc.declare_dram_parameter("srcLn", [128, NSLICE], bf16, isOutput=False)
    srcLp_d = nc.declare_dram_parameter("srcLp", [128, NSLICE], bf16, isOutput=False)
    srcR_d = nc.declare_dram_parameter("srcR", [128, NSLICE], bf16, isOutput=False)
    tgtLn_d = nc.declare_dram_parameter("tgtLn", [128, M], bf16, isOutput=False)
    tgtLp_d = nc.declare_dram_parameter("tgtLp", [128, M], bf16, isOutput=False)
    tgtR_d = nc.declare_dram_parameter("tgtR", [128, M], bf16, isOutput=False)
    out1_d = nc.declare_dram_parameter("out1", [128, T1], f32, isOutput=True)
    out2_d = nc.declare_dram_parameter("out2", [128, T2], f32, isOutput=True)

    with tile.TileContext(nc) as tc:
        with (
            tc.tile_pool(name="ins", bufs=1) as ins,
            tc.tile_pool(name="psum", bufs=4, space="PSUM") as psum,
            tc.tile_pool(name="cps", bufs=4 if PAIR_GP else 6) as cps,
            tc.tile_pool(name="scr", bufs=2 if PAIR_GP else 3) as scr,
            tc.tile_pool(name="accs", bufs=1) as accs,
        ):
            s_srcLn = ins.tile([128, NSLICE], bf16, tag="srcLn", name="s_srcLn")
            s_srcLp = ins.tile([128, NSLICE], bf16, tag="srcLp", name="s_srcLp")
            s_srcR = ins.tile([128, NSLICE], bf16, tag="srcR", name="s_srcR")
            s_tgtLn = ins.tile([128, M], bf16, tag="tgtLn", name="s_tgtLn")
            s_tgtLp = ins.tile([128, M], bf16, tag="tgtLp", name="s_tgtLp")
            s_tgtR = ins.tile([128, M], bf16, tag="tgtR", name="s_tgtR")

            # one DMA per tensor: a single InstDMACopy is internally split
            # across all 16 SDMA engines, and fewer DMAs keeps the sync-wait
            # count on consumer matmuls within the ISA limit.
            nc.sync.dma_start(out=s_srcLn[:, :], in_=srcLn_d[:, :])
            nc.sync.dma_start(out=s_tgtR[:, :], in_=tgtR_d[:, :])
            nc.sync.dma_start(out=s_srcLp[:, :], in_=srcLp_d[:, :])
            nc.sync.dma_start(out=s_tgtLn[:, :], in_=tgtLn_d[:, :])
            nc.sync.dma_start(out=s_srcR[:, :], in_=srcR_d[:, :])
            nc.sync.dma_start(out=s_tgtLp[:, :], in_=tgtLp_d[:, :])

            d1acc = accs.tile([128, T1 * NPART1], f32, tag="d1acc", name="d1acc")
            d2acc = accs.tile([128, T2 * NPART2], f32, tag="d2acc", name="d2acc")

            def mm_direct(lhsn, rhs_sb, lt, c0, pa, pb):
                # 4 chunks of -d2 into the two PSUM tiles
                for q in range(4):
                    dst = pa if q < 2 else pb
                    o = (q % 2) * 512
                    cc = (c0 + q) * 512
                    nc.tensor.matmul(
                        out=dst[:, o:o + 512],
                        lhsT=lhsn[q * 32: q * 32 + KROWS, lt],
                        rhs=rhs_sb[q * 32: q * 32 + KROWS, cc:cc + 512],
                        start=True, stop=True,
                        tile_position=(q * 32, 0),
                    )

            def mm_tourn(lhsn, lhsp, rhs_sb, lt, c0, pa, pb):
                # pa = [-d2(c0) | -d2(c2)], pb = [d2(c0)-d2(c1) | d2(c2)-d2(c3)]
                for i, q in enumerate((0, 1)):
                    cc = (c0 + 2 * i) * 512
                    nc.tensor.matmul(
                        out=pa[:, i * 512:(i + 1) * 512],
                        lhsT=lhsn[q * 32: q * 32 + KROWS, lt],
                        rhs=rhs_sb[q * 32: q * 32 + KROWS, cc:cc + 512],
                        start=True, stop=True,
                        tile_position=(q * 32, 0),
                    )
                for i, q in enumerate((2, 3)):
                    cc = (c0 + 2 * i) * 512
                    cd = (c0 + 2 * i + 1) * 512
                    nc.tensor.matmul(
                        out=pb[:, i * 512:(i + 1) * 512],
                        lhsT=lhsp[q * 32: q * 32 + KROWS, lt],
                        rhs=rhs_sb[q * 32: q * 32 + KROWS, cc:cc + 512],
                        start=True, stop=False,
                        tile_position=(q * 32, 0),
                    )
                    nc.tensor.matmul(
                        out=pb[:, i * 512:(i + 1) * 512],
                        lhsT=lhsn[q * 32: q * 32 + KROWS, lt],
                        rhs=rhs_sb[q * 32: q * 32 + KROWS, cd:cd + 512],
                        start=False, stop=True,
                        tile_position=(q * 32, 0),
                    )

            def tourn_drain_half(pa, pb, cpbig, rlbig, half):
                o = half * 1024
                nc.scalar.activation(out=rlbig[:, o:o + 1024], in_=pb, func=RELU)
                nc.scalar.copy(out=cpbig[:, o:o + 1024], in_=pa)

            def phase(lhsn, lhsp, rhs_sb, n_t, n_ss, accbuf, npart, direct_ss):
                n_tourn = n_ss - direct_ss
                pair_state = {}
                for t in range(n_t):
                    pc = t * npart
                    lt = slice(t * 128, (t + 1) * 128)
                    for ss in range(n_ss):
                        pa = psum.tile([128, 1024], f32, name="pa", tag="ps")
                        pb = psum.tile([128, 1024], f32, name="pb", tag="ps")
                        c0 = ss * 4
                        if ss < direct_ss:
                            mm_direct(lhsn, rhs_sb, lt, c0, pa, pb)
                            nc.vector.tensor_reduce(
                                out=accbuf[:, pc:pc + 1], in_=pa, axis=AXX, op=MAX)
                            nc.vector.tensor_reduce(
                                out=accbuf[:, pc + 1:pc + 2], in_=pb, axis=AXX, op=MAX)
                            pc += 2
                            continue
                        mm_tourn(lhsn, lhsp, rhs_sb, lt, c0, pa, pb)
                        if PAIR_GP and n_tourn == 1 and n_t % 2 == 0:
                            # pair the single tournament superstep across
                            # consecutive output-tiles t (even/odd halves)
                            half = t % 2
                            if half == 0:
                                cpbig = cps.tile([128, 2048], f32,
                                                 name="cpbig", tag="cp")
                                rlbig = cps.tile([128, 2048], f32,
                                                 name="rlbig", tag="cp")
                                pair_state["cpbig"] = cpbig
                                pair_state["rlbig"] = rlbig
                            else:
                                cpbig = pair_state["cpbig"]
                                rlbig = pair_state["rlbig"]
                            tourn_drain_half(pa, pb, cpbig, rlbig, half)
                            if half == 1:
                                sc = scr.tile([128, 2048], f32, name="sc")
                                nc.gpsimd.tensor_tensor(
                                    out=sc, in0=cpbig, in1=rlbig, op=ADD)
                                accv = accbuf.rearrange(
                                    "p (t s) -> p t s", s=npart)
                                off = pc - t * npart
                                nc.vector.tensor_reduce(
                                    out=accv[:, t - 1:t + 1, off:off + 1],
                                    in_=sc.rearrange("p (h c) -> p h c", c=1024),
                                    axis=AXX, op=MAX)
                            pc += 1
                            continue
                        if not (PAIR_GP and n_tourn % 2 == 0):
                            rl = cps.tile([128, 1024], f32, name="rl", tag="cp")
                            nc.scalar.activation(out=rl, in_=pb, func=RELU)
                            cp = cps.tile([128, 1024], f32, name="cp", tag="cp")
                            nc.scalar.copy(out=cp, in_=pa)
                            sc = scr.tile([128, 1024], f32, name="sc")
                            nc.gpsimd.tensor_tensor(out=sc, in0=cp, in1=rl, op=ADD)
                            nc.vector.tensor_reduce(
                                out=accbuf[:, pc:pc + 1], in_=sc, axis=AXX, op=MAX)
                            pc += 1
                            continue
                        # paired tournament: two supersteps share one wide
                        # GpSimd add and one wide VectorE reduce
                        half = (ss - direct_ss) % 2
                        if half == 0:
                            cpbig = cps.tile([128, 2048], f32, name="cpbig", tag="cp")
                            rlbig = cps.tile([128, 2048], f32, name="rlbig", tag="cp")
                        tourn_drain_half(pa, pb, cpbig, rlbig, half)
                        if half == 1:
                            sc = scr.tile([128, 2048], f32, name="sc")
                            nc.gpsimd.tensor_tensor(out=sc, in0=cpbig, in1=rlbig, op=ADD)
                            nc.vector.tensor_reduce(
                                out=accbuf[:, pc:pc + 1], in_=sc, axis=AXX, op=MAX)
                            pc += 1

            def whole_body():
                phase(s_srcLn, s_srcLp, s_tgtR, T1, SS1, d1acc, NPART1, D1_DIRECT)
                phase(s_tgtLn, s_tgtLp, s_srcR, T2, SS2, d2acc, NPART2, D2_DIRECT)

                o1 = accs.tile([128, T1], f32, tag="o1", name="o1")
                nc.vector.tensor_reduce(
                    out=o1,
                    in_=d1acc.rearrange("p (t s) -> p t s", s=NPART1),
                    axis=AXX,
                    op=MAX,
                )
                nc.sync.dma_start(out=out1_d[:, :], in_=o1)

                o2 = accs.tile([128, T2], f32, tag="o2", name="o2")
                nc.vector.tensor_reduce(
                    out=o2,
                    in_=d2acc.rearrange("p (t s) -> p t s", s=NPART2),
                    axis=AXX,
                    op=MAX,
                )
                nc.sync.dma_start(out=out2_d[:, :], in_=o2)

            if repeat == 1:
                whole_body()
            else:
                with tc.For_i(0, repeat, 1):
                    whole_body()

    if not nc.is_finalized():
        nc.finalize()
    _NC_CACHE[repeat] = nc
    return nc


def make_in_maps(source_pc, target_pc):
    """Host-side prep: per-core dicts of quadrant-replicated bf16 forms.

    The 'Ln' arrays are the exact negation of the lhs form, so the PE
    emits -d2; 'Lp' is the positive lhs form used for the tournament
    difference tiles.
    """
    source_pc = np.asarray(source_pc, dtype=np.float32)
    target_pc = np.asarray(target_pc, dtype=np.float32)
    tgt_quads = []
    for b in range(B):
        tl, tr = _forms(target_pc[b])
        tgt_quads.append((_quad(_neg(tl)), _quad(tl), _quad(tr)))
    in_maps = []
    for c in range(CORES):
        b, qq = divmod(c, 4)
        src_slice = source_pc[b][qq * NSLICE: (qq + 1) * NSLICE]
        sl, sr = _forms(src_slice)
        tln, tlp, tr = tgt_quads[b]
        in_maps.append({
            "srcLn": _quad(_neg(sl)),
            "srcLp": _quad(sl),
            "srcR": _quad(sr),
            "tgtLn": tln,
            "tgtLp": tlp,
            "tgtR": tr,
        })
    return in_maps


def postprocess(results):
    """Combine per-core outputs into the [B] chamfer distances.

    Device outputs are max(-d2) partials, i.e. negated squared mins.
    """
    out = np.zeros(B, dtype=np.float32)
    for b in range(B):
        d1sq = -np.concatenate(
            [results[b * 4 + q]["out1"].T.reshape(-1) for q in range(4)]
        )
        d2sq = -np.max(
            np.stack([results[b * 4 + q]["out2"].T.reshape(-1) for q in range(4)]),
            axis=0,
        )
        d1 = np.minimum(np.sqrt(np.maximum(d1sq, 0.0)), THRESHOLD).mean(
            dtype=np.float64
        )
        d2 = np.minimum(np.sqrt(np.maximum(d2sq, 0.0)), THRESHOLD).mean(
            dtype=np.float64
        )
        out[b] = 0.5 * (d1 + d2)
    return out


def kernel(source_pc, target_pc):
    from concourse.bass_utils import run_bass_kernel_spmd

    nc = build_bass()
    in_maps = make_in_maps(source_pc, target_pc)
    res = run_bass_kernel_spmd(nc, in_maps, list(range(CORES))).results
    return postprocess(res)


# revision 16
# speedup vs baseline: 1.1070x; 1.1070x over previous
"""Chamfer distance (bidirectional, thresholded) on 8 Trainium2 NeuronCores.

Problem: source_pc/target_pc [2, 16384, 3] fp32 -> [2] fp32.
  dist[b,n,m] = ||src[b,n] - tgt[b,m]||
  out[b] = (mean_n min(min_m dist, T) + mean_m min(min_n dist, T)) / 2

Strategy
--------
Sharding: batch (2) x source-slice (4) over the 8 cores. Each core computes,
for its batch b and its 4096-point source slice:
  * dist1 partial: min_m d2(n, m) for its 4096 n's (complete min over all m)
  * dist2 partial: min_{n in slice} d2(n, m) for all 16384 m's
The host min-reduces dist2 partials across the 4 cores of a batch and applies
sqrt/threshold/mean (cheap: 16K values).

Device kernel: d2 is computed by the TensorEngine via an augmented inner
product of K=30 rows: d2 = x^2 + y^2 - 2xy, with each fp32 operand split into
3 bf16 chunks (exact 24-bit split; cross-chunk product rows give fp32-level
accuracy at bf16 PE speed - fp32 matmul is 4x slower). K=30 <= 32 enables 4x
row-tiling: 4 concurrent matmuls via tile_position=(32q, 0), each fed from
its own 32-partition SBUF quadrant (inputs quadrant-replicated on host).

The bottleneck is the min-reduction: 2*2*16384^2 = 1.07G d2 values must
leave PSUM, and VectorE is the only engine with a min-reduce (1 col/cycle;
the fused tensor_tensor_reduce and Pool-engine min are rejected by this
toolchain). To beat the DVE-only rate the kernel works on NEGATED d2 and
splits supersteps into two kinds:
  * direct: VectorE max-reduces the two PSUM tiles (max of -d2 = -min d2).
  * tournament: the PE additionally emits D' = d2(c_i) - d2(c_{i+1}) via a
    2-matmul PSUM accumulation; ScalarE computes relu(D') and copies A'
    (both PSUM->SBUF, line rate); GpSimd adds them (the only elementwise op
    walrus accepts on Pool): A' + relu(D') = max(-d2(c_i), -d2(c_{i+1})),
    halving what VectorE must reduce for those supersteps.
This spreads the drain across ScalarE + GpSimd + VectorE at ~1.3x the
DVE-only throughput.
"""

import numpy as np
import ml_dtypes

B = 2
N = 16384
M = 16384
CORES = 8
NSLICE = N // 4          # source points per core
T1 = NSLICE // 128       # 32 dist1 output tiles (n on partitions)
T2 = M // 128            # 128 dist2 output tiles (m on partitions)
SS1 = M // 2048          # 8 supersteps (of 4x512 streamed cols) per n-tile
SS2 = NSLICE // 2048     # 2 supersteps per m-tile
# supersteps whose two PSUM tiles VectorE max-reduces directly; the rest go
# through the ScalarE-relu / GpSimd-add tournament (1 partial instead of 2)
D1_DIRECT = 4
D2_DIRECT = 1
# pair two tournament supersteps into one wide [128,2048] GpSimd add + one
# wide VectorE reduce (GpSimd per-op overhead is the tournament binder)
PAIR_GP = False
# spread direct supersteps evenly among tournament ones (off = verified config)
INTERLEAVE = False
def _npart(n_ss, direct):
    n_tourn = n_ss - direct
    if PAIR_GP and n_tourn % 2 == 0:
        return 2 * direct + n_tourn // 2
    # n_tourn == 1: paired across consecutive output-tiles instead
    return 2 * direct + n_tourn


NPART1 = _npart(SS1, D1_DIRECT)
NPART2 = _npart(SS2, D2_DIRECT)
KROWS = 30
THRESHOLD = 33.33

# chunk-index pairs (lhs_chunk, rhs_chunk); 0=hi 1=mid 2=lo. (2,2) dropped
# (contributes ~2^-34 relative - far below fp32 rounding of the sum).
_PAIRS = [(0, 0), (0, 1), (1, 0), (0, 2), (2, 0), (1, 1), (1, 2), (2, 1)]

_BF16 = ml_dtypes.bfloat16


def _split3(a):
    """Exact 3-way bf16 split of fp32: a == h + m + l (24-bit mantissa)."""
    h = a.astype(_BF16)
    r = a - h.astype(np.float32)
    m = r.astype(_BF16)
    r2 = r - m.astype(np.float32)
    l = r2.astype(_BF16)
    return h, m, l


def _forms(pts):
    """pts [n,3] fp32 -> (lhs_form, rhs_form), each [KROWS, n] bf16.

    sum_k lhs[k, i] * rhs'[k, j] (for rhs' built from another point set)
    = |p_i|^2 + |q_j|^2 - 2 p_i . q_j  (up to dropped (lo,lo) terms).
    """
    pts = np.ascontiguousarray(pts, dtype=np.float32)
    n = pts.shape[0]
    sq = np.sum(pts * pts, axis=1, dtype=np.float32)
    coord_l = [_split3(np.float32(-2.0) * pts[:, d]) for d in range(3)]
    coord_r = [_split3(pts[:, d]) for d in range(3)]
    sq_c = _split3(sq)
    ones = np.ones(n, dtype=_BF16)
    lhs = np.empty((KROWS, n), dtype=_BF16)
    rhs = np.empty((KROWS, n), dtype=_BF16)
    k = 0
    for d in range(3):
        for (i, j) in _PAIRS:
            lhs[k] = coord_l[d][i]
            rhs[k] = coord_r[d][j]
            k += 1
    for c in range(3):
        lhs[k] = sq_c[c]
        rhs[k] = ones
        k += 1
    for c in range(3):
        lhs[k] = ones
        rhs[k] = sq_c[c]
        k += 1
    assert k == KROWS
    return lhs, rhs


def _quad(a):
    """[KROWS, X] -> [128, X]: replicate into the 4 SBUF quadrants."""
    out = np.zeros((128, a.shape[1]), dtype=a.dtype)
    for q in range(4):
        out[q * 32: q * 32 + KROWS] = a
    return out


def _neg(a):
    """Exact bf16 negation."""
    return (-a.astype(np.float32)).astype(_BF16)


_NC_CACHE = {}


def build_bass(repeat=1):
    """Build (and cache) the single-core Bass/Tile program.

    repeat > 1 wraps the whole compute in an on-device loop; used by the
    test harness to amortize the ~88 ms axon dispatch floor when timing.
    """
    if repeat in _NC_CACHE:
        return _NC_CACHE[repeat]

    import concourse.tile as tile
    from concourse import bacc, mybir

    f32 = mybir.dt.float32
    bf16 = mybir.dt.bfloat16
    MAX = mybir.AluOpType.max
    ADD = mybir.AluOpType.add
    AXX = mybir.AxisListType.X
    RELU = mybir.ActivationFunctionType.Relu

    nc = bacc.Bacc(None, target_bir_lowering=False)
    srcLn_d = nc.declare_dram_parameter("srcLn", [128, NSLICE], bf16, isOutput=False)
    srcLp_d = nc.declare_dram_parameter("srcLp", [128, NSLICE], bf16, isOutput=False)
    srcR_d = nc.declare_dram_parameter("srcR", [128, NSLICE], bf16, isOutput=False)
    tgtLn_d = nc.declare_dram_parameter("tgtLn", [128, M], bf16, isOutput=False)
    tgtLp_d = nc.declare_dram_parameter("tgtLp", [128, M], bf16, isOutput=False)
    tgtR_d = nc.declare_dram_parameter("tgtR", [128, M], bf16, isOutput=False)
    out1_d = nc.declare_dram_parameter("out1", [128, T1], f32, isOutput=True)
    out2_d = nc.declare_dram_parameter("out2", [128, T2], f32, isOutput=True)

    with tile.TileContext(nc) as tc:
        with (
            tc.tile_pool(name="ins", bufs=1) as ins,
            tc.tile_pool(name="psum", bufs=4, space="PSUM") as psum,
            tc.tile_pool(name="cps", bufs=4 if PAIR_GP else 6) as cps,
            tc.tile_pool(name="scr", bufs=2 if PAIR_GP else 3) as scr,
            tc.tile_pool(name="accs", bufs=1) as accs,
        ):
            s_srcLn = ins.tile([128, NSLICE], bf16, tag="srcLn", name="s_srcLn")
            s_srcLp = ins.tile([128, NSLICE], bf16, tag="srcLp", name="s_srcLp")
            s_srcR = ins.tile([128, NSLICE], bf16, tag="srcR", name="s_srcR")
            s_tgtLn = ins.tile([128, M], bf16, tag="tgtLn", name="s_tgtLn")
            s_tgtLp = ins.tile([128, M], bf16, tag="tgtLp", name="s_tgtLp")
            s_tgtR = ins.tile([128, M], bf16, tag="tgtR", name="s_tgtR")

            # one DMA per tensor: a single InstDMACopy is internally split
            # across all 16 SDMA engines, and fewer DMAs keeps the sync-wait
            # count on consumer matmuls within the ISA limit.
            nc.sync.dma_start(out=s_srcLn[:, :], in_=srcLn_d[:, :])
            nc.sync.dma_start(out=s_tgtR[:, :], in_=tgtR_d[:, :])
            nc.sync.dma_start(out=s_srcLp[:, :], in_=srcLp_d[:, :])
            nc.sync.dma_start(out=s_tgtLn[:, :], in_=tgtLn_d[:, :])
            nc.sync.dma_start(out=s_srcR[:, :], in_=srcR_d[:, :])
            nc.sync.dma_start(out=s_tgtLp[:, :], in_=tgtLp_d[:, :])

            d1acc = accs.tile([128, T1 * NPART1], f32, tag="d1acc", name="d1acc")
            d2acc = accs.tile([128, T2 * NPART2], f32, tag="d2acc", name="d2acc")

            def mm_direct(lhsn, rhs_sb, lt, c0, pa, pb):
                # 4 chunks of -d2 into the two PSUM tiles
                for q in range(4):
                    dst = pa if q < 2 else pb
                    o = (q % 2) * 512
                    cc = (c0 + q) * 512
                    nc.tensor.matmul(
                        out=dst[:, o:o + 512],
                        lhsT=lhsn[q * 32: q * 32 + KROWS, lt],
                        rhs=rhs_sb[q * 32: q * 32 + KROWS, cc:cc + 512],
                        start=True, stop=True,
                        tile_position=(q * 32, 0),
                    )

            def mm_tourn(lhsn, lhsp, rhs_sb, lt, c0, pa, pb):
                # pa = [-d2(c0) | -d2(c2)], pb = [d2(c0)-d2(c1) | d2(c2)-d2(c3)]
                for i, q in enumerate((0, 1)):
                    cc = (c0 + 2 * i) * 512
                    nc.tensor.matmul(
                        out=pa[:, i * 512:(i + 1) * 512],
                        lhsT=lhsn[q * 32: q * 32 + KROWS, lt],
                        rhs=rhs_sb[q * 32: q * 32 + KROWS, cc:cc + 512],
                        start=True, stop=True,
                        tile_position=(q * 32, 0),
                    )
                for i, q in enumerate((2, 3)):
                    cc = (c0 + 2 * i) * 512
                    cd = (c0 + 2 * i + 1) * 512
                    nc.tensor.matmul(
                        out=pb[:, i * 512:(i + 1) * 512],
                        lhsT=lhsp[q * 32: q * 32 + KROWS, lt],
                        rhs=rhs_sb[q * 32: q * 32 + KROWS, cc:cc + 512],
                        start=True, stop=False,
                        tile_position=(q * 32, 0),
                    )
                    nc.tensor.matmul(
                        out=pb[:, i * 512:(i + 1) * 512],
                        lhsT=lhsn[q * 32: q * 32 + KROWS, lt],
                        rhs=rhs_sb[q * 32: q * 32 + KROWS, cd:cd + 512],
                        start=False, stop=True,
                        tile_position=(q * 32, 0),
                    )

            def tourn_drain_half(pa, pb, cpbig, rlbig, half):
                o = half * 1024
                nc.scalar.activation(out=rlbig[:, o:o + 1024], in_=pb, func=RELU)
                nc.scalar.copy(out=cpbig[:, o:o + 1024], in_=pa)

            def phase(lhsn, lhsp, rhs_sb, n_t, n_ss, accbuf, npart, direct_ss):
                n_tourn = n_ss - direct_ss
                pair_state = {}
                stride = n_ss // direct_ss if direct_ss else 0
                for t in range(n_t):
                    pc = t * npart
                    lt = slice(t * 128, (t + 1) * 128)
                    nd = 0
                    for ss in range(n_ss):
                        pa = psum.tile([128, 1024], f32, name="pa", tag="ps")
                        pb = psum.tile([128, 1024], f32, name="pb", tag="ps")
                        c0 = ss * 4
                        if INTERLEAVE:
                            is_direct = (direct_ss > 0 and ss % stride == 0
                                         and nd < direct_ss)
                        else:
                            is_direct = ss < direct_ss
                        if is_direct:
                            nd += 1
                            mm_direct(lhsn, rhs_sb, lt, c0, pa, pb)
                            nc.vector.tensor_reduce(
                                out=accbuf[:, pc:pc + 1], in_=pa, axis=AXX, op=MAX)
                            nc.vector.tensor_reduce(
                                out=accbuf[:, pc + 1:pc + 2], in_=pb, axis=AXX, op=MAX)
                            pc += 2
                            continue
                        mm_tourn(lhsn, lhsp, rhs_sb, lt, c0, pa, pb)
                        if PAIR_GP and n_tourn == 1 and n_t % 2 == 0:
                            # pair the single tournament superstep across
                            # consecutive output-tiles t (even/odd halves)
                            half = t % 2
                            if half == 0:
                                cpbig = cps.tile([128, 2048], f32,
                                                 name="cpbig", tag="cp")
                                rlbig = cps.tile([128, 2048], f32,
                                                 name="rlbig", tag="cp")
                                pair_state["cpbig"] = cpbig
                                pair_state["rlbig"] = rlbig
                            else:
                                cpbig = pair_state["cpbig"]
                                rlbig = pair_state["rlbig"]
                            tourn_drain_half(pa, pb, cpbig, rlbig, half)
                            if half == 1:
                                sc = scr.tile([128, 2048], f32, name="sc")
                                nc.gpsimd.tensor_tensor(
                                    out=sc, in0=cpbig, in1=rlbig, op=ADD)
                                accv = accbuf.rearrange(
                                    "p (t s) -> p t s", s=npart)
                                off = pc - t * npart
                                nc.vector.tensor_reduce(
                                    out=accv[:, t - 1:t + 1, off:off + 1],
                                    in_=sc.rearrange("p (h c) -> p h c", c=1024),
                                    axis=AXX, op=MAX)
                            pc += 1
                            continue
                        if not (PAIR_GP and n_tourn % 2 == 0):
                            rl = cps.tile([128, 1024], f32, name="rl", tag="cp")
                            nc.scalar.activation(out=rl, in_=pb, func=RELU)
                            cp = cps.tile([128, 1024], f32, name="cp", tag="cp")
                            nc.scalar.copy(out=cp, in_=pa)
                            sc = scr.tile([128, 1024], f32, name="sc")
                            nc.gpsimd.tensor_tensor(out=sc, in0=cp, in1=rl, op=ADD)
                            nc.vector.tensor_reduce(
                                out=accbuf[:, pc:pc + 1], in_=sc, axis=AXX, op=MAX)
                            pc += 1
                            continue
                        # paired tournament: two supersteps share one wide
                        # GpSimd add and one wide VectorE reduce
                        half = (ss - direct_ss) % 2
                        if half == 0:
                            cpbig = cps.tile([128, 2048], f32, name="cpbig", tag="cp")
                            rlbig = cps.tile([128, 2048], f32, name="rlbig", tag="cp")
                        tourn_drain_half(pa, pb, cpbig, rlbig, half)
                        if half == 1:
                            sc = scr.tile([128, 2048], f32, name="sc")
                            nc.gpsimd.tensor_tensor(out=sc, in0=cpbig, in1=rlbig, op=ADD)
                            nc.vector.tensor_reduce(
                                out=accbuf[:, pc:pc + 1], in_=sc, axis=AXX, op=MAX)
                            pc += 1

            def whole_body():
                phase(s_srcLn, s_srcLp, s_tgtR, T1, SS1, d1acc, NPART1, D1_DIRECT)
                phase(s_tgtLn, s_tgtLp, s_srcR, T2, SS2, d2acc, NPART2, D2_DIRECT)

                o1 = accs.tile([128, T1], f32, tag="o1", name="o1")
                nc.vector.tensor_reduce(
                    out=o1,
                    in_=d1acc.rearrange("p (t s) -> p t s", s=NPART1),
                    axis=AXX,
                    op=MAX,
                )
                nc.sync.dma_start(out=out1_d[:, :], in_=o1)

                o2 = accs.tile([128, T2], f32, tag="o2", name="o2")
                nc.vector.tensor_reduce(
                    out=o2,
                    in_=d2acc.rearrange("p (t s) -> p t s", s=NPART2),
                    axis=AXX,
                    op=MAX,
                )
                nc.sync.dma_start(out=out2_d[:, :], in_=o2)

            if repeat == 1:
                whole_body()
            else:
                with tc.For_i(0, repeat, 1):
                    whole_body()

    if not nc.is_finalized():
        nc.finalize()
    _NC_CACHE[repeat] = nc
    return nc


def make_in_maps(source_pc, target_pc):
    """Host-side prep: per-core dicts of quadrant-replicated bf16 forms.

    The 'Ln' arrays are the exact negation of the lhs form, so the PE
    emits -d2; 'Lp' is the positive lhs form used for the tournament
    difference tiles.
    """
    source_pc = np.asarray(source_pc, dtype=np.float32)
    target_pc = np.asarray(target_pc, dtype=np.float32)
    tgt_quads = []
    for b in range(B):
        tl, tr = _forms(target_pc[b])
        tgt_quads.append((_quad(_neg(tl)), _quad(tl), _quad(tr)))
    in_maps = []
    for c in range(CORES):
        b, qq = divmod(c, 4)
        src_slice = source_pc[b][qq * NSLICE: (qq + 1) * NSLICE]
        sl, sr = _forms(src_slice)
        tln, tlp, tr = tgt_quads[b]
        in_maps.append({
            "srcLn": _quad(_neg(sl)),
            "srcLp": _quad(sl),
            "srcR": _quad(sr),
            "tgtLn": tln,
            "tgtLp": tlp,
            "tgtR": tr,
        })
    return in_maps


def postprocess(results):
    """Combine per-core outputs into the [B] chamfer distances.

    Device outputs are max(-d2) partials, i.e. negated squared mins.
    """
    out = np.zeros(B, dtype=np.float32)
    for b in range(B):
        d1sq = -np.concatenate(
            [results[b * 4 + q]["out1"].T.reshape(-1) for q in range(4)]
        )
        d2sq = -np.max(
            np.stack([results[b * 4 + q]["out2"].T.reshape(-1) for q in range(4)]),
            axis=0,
        )
        d1 = np.minimum(np.sqrt(np.maximum(d1sq, 0.0)), THRESHOLD).mean(
            dtype=np.float64
        )
        d2 = np.minimum(np.sqrt(np.maximum(d2sq, 0.0)), THRESHOLD).mean(
            dtype=np.float64
        )
        out[b] = 0.5 * (d1 + d2)
    return out


def kernel(source_pc, target_pc):
    from concourse.bass_utils import run_bass_kernel_spmd

    nc = build_bass()
    in_maps = make_in_maps(source_pc, target_pc)
    res = run_bass_kernel_spmd(nc, in_maps, list(range(CORES))).results
    return postprocess(res)
